# revision 1
# baseline (speedup 1.0000x reference)
"""CertViT (ViT-Base + layer-3 token pruning) forward pass on 8 Trainium2 cores.

Data parallel: 8 images per core as 4 image-pairs. v2: bf16 end-to-end.
Activations live in channel-partition layout x^T [768 -> 6x128 chunks, tokens],
residual stream in bf16; PSUM accumulation fp32. LayerNorm affine params are
folded into the following matmul weights on the host; q-scale folded into the
q weights; v-bias folded into the proj bias. Attention QK/AV run per-image
(bf16 is full-rate at any free dim, unlike fp32r which needs >=256), with
even/odd heads packed into one PSUM bank via column tile_position so the
softmax-normalize evacuation is a single op per head pair. Softmax and LN
reciprocals use the ~5x-faster reciprocal_approx_fast custom DVE op. Top-k
pruning uses max8/match_replace for the drop mask, a triangular-matmul cumsum
for ranks, and a one-hot permutation matmul for the gather.
"""

import os
import sys

import numpy as np

for _p in ('/opt/trn_rl_repo', '/root/.axon_site/_ro/trn_rl_repo'):
    if os.path.isdir(_p) and _p not in sys.path:
        sys.path.append(_p)

import ml_dtypes
import concourse.bass as bass
import concourse.mybir as mybir
from concourse.tile import TileContext
from concourse.bass_utils import run_bass_kernel_spmd
from concourse.alu_op_type import AluOpType as AL

dt = mybir.dt
AF = mybir.ActivationFunctionType
BF16 = ml_dtypes.bfloat16

# ---------------------------------------------------------------- config
NCORES = 8
B_CORE = 8            # images per core
PAIRS = B_CORE // 2
C = 768
CH = C // 128          # 6 channel chunks
HD = 12                # heads
D = 64                 # head dim
SCALE = D ** -0.5
DEPTH = 12
SEL = 3                # pruning layer
N0 = 197               # tokens before pruning
K_KEEP = 137           # int(197*0.7)
N_DROP = N0 - 1 - K_KEEP   # 59
N1 = K_KEEP + 2        # 139 tokens after pruning
F0 = 2 * N0            # pair free dim, layers 0..3
F1 = 2 * N1            # pair free dim, layers 4..11
EPS = 1e-6
NCLS = 100

# ------------------------------------------------------------- waitfix
# This walrus build accepts at most ONE sem wait per instruction; Tile can
# attach several. Move excess waits onto InstNoOp carriers inserted before.
_wf_counter = [0]


def _wf_carrier(engine, waits):
    _wf_counter[0] += 1
    d = mybir.InstNoOp(name=f"waitfix-{_wf_counter[0]}", ins=[], outs=[])
    d.engine = engine
    d.sync_info = mybir.SyncInfo(on_wait=list(waits), on_update=[])
    return d


def split_excess_waits(nc, max_waits=1):
    nfix = 0
    for f in nc.m.functions:
        for bb in f.blocks:
            insts = list(bb.instructions)
            out = []
            changed = False
            for inst in insts:
                si = inst.sync_info
                waits = list(si.on_wait) if si and si.on_wait else []
                if len(waits) > max_waits:
                    keep, rest = waits[:max_waits], waits[max_waits:]
                    while rest:
                        chunk, rest = rest[:max_waits], rest[max_waits:]
                        out.append(_wf_carrier(inst.engine, chunk))
                    si.on_wait = keep
                    changed = True
                    nfix += 1
                out.append(inst)
            if changed:
                bb.instructions = out
    return nfix


# ----------------------------------------------------------- device kernel
def build_nc():
    nc = bass.Bass()
    f32, bf16 = dt.float32, dt.bfloat16

    d = {}
    d["patches_d"] = nc.declare_dram_parameter("patchesT", [C, B_CORE * 196], bf16, isOutput=False)
    d["posc_d"] = nc.declare_dram_parameter("posCT", [C, N0], f32, isOutput=False)
    d["pw_d"] = nc.declare_dram_parameter("patch_wT", [C, C], bf16, isOutput=False)
    d["qkvw_d"] = nc.declare_dram_parameter("qkv_wT", [DEPTH, C, 3 * C], bf16, isOutput=False)
    d["qkvb_d"] = nc.declare_dram_parameter("qkv_bL", [DEPTH, 128, 12], f32, isOutput=False)
    d["projw_d"] = nc.declare_dram_parameter("proj_wT", [DEPTH, C, C], bf16, isOutput=False)
    d["projb_d"] = nc.declare_dram_parameter("proj_bL", [DEPTH, 128, 6], f32, isOutput=False)
    d["fc1w_d"] = nc.declare_dram_parameter("fc1_wT", [DEPTH, C, 4 * C], bf16, isOutput=False)
    d["fc1b_d"] = nc.declare_dram_parameter("fc1_bL", [DEPTH, 128, 24], f32, isOutput=False)
    d["fc2w_d"] = nc.declare_dram_parameter("fc2_wT", [DEPTH, 4 * C, C], bf16, isOutput=False)
    d["fc2b_d"] = nc.declare_dram_parameter("fc2_bL", [DEPTH, 128, 6], f32, isOutput=False)
    d["headw_d"] = nc.declare_dram_parameter("headT", [C, NCLS], bf16, isOutput=False)
    d["headb_d"] = nc.declare_dram_parameter("head_bL", [NCLS, 1], f32, isOutput=False)
    d["identb_d"] = nc.declare_dram_parameter("identb", [128, 128], bf16, isOutput=False)
    d["identf_d"] = nc.declare_dram_parameter("identf", [128, 128], f32, isOutput=False)
    d["onesb_d"] = nc.declare_dram_parameter("onesb", [128, 128], bf16, isOutput=False)
    d["onesr_d"] = nc.declare_dram_parameter("onesr", [128, 128], dt.float32r, isOutput=False)
    d["iota_d"] = nc.declare_dram_parameter("iota", [128, N1 - 1], bf16, isOutput=False)
    d["lt_d"] = nc.declare_dram_parameter("LT", [196, 196], bf16, isOutput=False)
    d["out_d"] = nc.declare_dram_parameter("logitsT", [NCLS, B_CORE], f32, isOutput=True)

    d["dbg_layer"] = os.environ.get("BASS_VIT_DEBUG_LAYER", "")
    if d["dbg_layer"]:
        d["dbg_d"] = nc.declare_dram_parameter("dbg", [1 + 2 * DEPTH, 128, CH * F0], f32, isOutput=True)
        d["dbgp_d"] = nc.declare_dram_parameter("dbgp", [4, 8, 196], f32, isOutput=True)
    else:
        d["dbg_d"] = None
        d["dbgp_d"] = None

    with TileContext(nc) as tc:
        _build_body(nc, tc, d)
    return nc


def _build_body(nc, tc, d):
    f32, f32r, bf16 = dt.float32, dt.float32r, dt.bfloat16
    from contextlib import ExitStack
    es = ExitStack()

    cpool = es.enter_context(tc.tile_pool(name="consts", bufs=1))
    xpool = es.enter_context(tc.tile_pool(name="x", bufs=1))
    ppool = es.enter_context(tc.tile_pool(name="psum", bufs=1, space="PSUM"))
    prpool = es.enter_context(tc.tile_pool(name="prune", bufs=1))
    bpool = es.enter_context(tc.tile_pool(name="bias", bufs=2))
    wA = es.enter_context(tc.tile_pool(name="wA", bufs=1))
    wB = es.enter_context(tc.tile_pool(name="wB", bufs=1))

    # constants
    identb = cpool.tile([128, 128], bf16, tag="identb")
    identf = cpool.tile([128, 128], f32, tag="identf")
    onesb = cpool.tile([128, 128], bf16, tag="onesb")
    onesr = cpool.tile([128, 128], f32r, tag="onesr")
    iota = cpool.tile([128, N1 - 1], bf16, tag="iota")
    ltt = cpool.tile([128, 2 * 196], bf16, tag="ltt")
    eps_t = cpool.tile([128, 1], f32, tag="eps_t")
    nc.vector.memset(eps_t[:], EPS)
    nc.sync.dma_start(identb[:], d["identb_d"][:])
    nc.sync.dma_start(identf[:], d["identf_d"][:])
    nc.sync.dma_start(onesb[:], d["onesb_d"][:])
    nc.sync.dma_start(onesr[:], d["onesr_d"][:])
    nc.sync.dma_start(iota[:], d["iota_d"][:])
    nc.sync.dma_start(ltt[:, 0:196], d["lt_d"][0:128, :])
    nc.sync.dma_start(ltt[0:68, 196:392], d["lt_d"][128:196, :])

    # PSUM slots: 4 tags x 2 bufs = 8 banks
    def psA():       # gemm accumulators (ln stats, qkv, v, proj, mlp)
        return ppool.tile([128, F0], f32, tag="a", bufs=2, name="psA")

    def psS():       # attention scores
        return ppool.tile([128, F0], f32, tag="sc", bufs=2, name="psS")

    def psV():       # attention AV
        return ppool.tile([128, F0], f32, tag="av", bufs=2, name="psV")

    def psD():       # softmax denominators (+ prune/head misc)
        return ppool.tile([128, F0], f32, tag="dn", bufs=2, name="psD")

    # persistent per-pair residual stream x^T, chunk-major [128, CH*F] f32r
    # (fp32 residual keeps the error budget; matmul inputs are bf16)
    xt = [xpool.tile([128, CH * F0], f32r, tag=f"x{p}", name=f"x{p}") for p in range(PAIRS)]
    # per-pair uncertainty rows (filled at layer SEL)
    unc = [prpool.tile([1, F0], f32, tag=f"unc{p}", name=f"unc{p}") for p in range(PAIRS)]
    # U rows for the prune top-k, prefilled during phase A of layer SEL
    U = prpool.tile([B_CORE, 196], f32, tag="U")

    # ------------------------------------------------------------ patch embed
    with tc.tile_pool(name="wpatch", bufs=1) as wp, tc.tile_pool(name="tpatch", bufs=2) as tp:
        posct = wp.tile([128, CH * N0], f32, tag="posct")
        nc.sync.dma_start(posct[:].rearrange("p (k n) -> p k n", k=CH), d["posc_d"].rearrange("(k p) n -> p k n", p=128))
        pwt = wp.tile([128, CH * C], bf16, tag="pw")
        nc.sync.dma_start(pwt[:].rearrange("p (k n) -> p k n", k=CH), d["pw_d"].rearrange("(k p) n -> p k n", p=128))
        for p in range(PAIRS):
            prt = tp.tile([128, CH * 392], bf16, tag="patches")
            nc.sync.dma_start(
                prt[:].rearrange("p (k n) -> p k n", k=CH),
                d["patches_d"][:, p * 392:(p + 1) * 392].rearrange("(k p) n -> p k n", p=128),
            )
            for co in range(CH):
                ps = psA()
                for k in range(CH):
                    nc.tensor.matmul(
                        ps[:, 0:392],
                        pwt[:, k * C + co * 128: k * C + co * 128 + 128],
                        prt[:, k * 392:(k + 1) * 392],
                        start=(k == 0), stop=(k == CH - 1),
                    )
                for b in range(2):
                    nc.vector.tensor_tensor(
                        xt[p][:, co * F0 + b * N0 + 1: co * F0 + b * N0 + N0],
                        ps[:, b * 196:(b + 1) * 196],
                        posct[:, co * N0 + 1: co * N0 + N0],
                        op=AL.add,
                    )
                    nc.vector.tensor_copy(
                        xt[p][:, co * F0 + b * N0: co * F0 + b * N0 + 1],
                        posct[:, co * N0: co * N0 + 1],
                    )

    def tap(slot, xtile, F):
        if d["dbg_d"] is not None:
            nc.sync.dma_start(d["dbg_d"][slot][:, 0:CH * F], xtile[:, 0:CH * F].bitcast(f32))

    tap(0, xt[0], F0)

    # ------------------------------------------------------------ helpers
    def layernorm(pool, x, F, xh_tag, xh_bufs=1, xf32=True, xh_pool=None):
        """Standardize x (chunk-major [128, CH*F]) per token -> bf16 tile.
        x is f32r (bitcast to f32 for DVE) when xf32, else bf16. Temps come
        from `pool`; the xh output from `xh_pool` (default: pool)."""
        xh = (xh_pool or pool).tile([128, CH * F], bf16, tag=xh_tag,
                                    bufs=xh_bufs, name=xh_tag)
        ones_s = onesr if xf32 else onesb

        def xk(k):
            s = x[:, k * F:(k + 1) * F]
            return s.bitcast(f32) if xf32 else s

        sq = pool.tile([128, CH * F], bf16, tag="ln_sq", bufs=1)
        for k in range(CH):
            nc.vector.tensor_tensor(
                sq[:, k * F:(k + 1) * F], xk(k), xk(k), op=AL.mult)
        pm = psD()
        ps2 = psD()
        for k in range(CH):
            nc.tensor.matmul(pm[:, 0:F], ones_s[:], x[:, k * F:(k + 1) * F],
                             start=(k == 0), stop=(k == CH - 1))
        for k in range(CH):
            nc.tensor.matmul(ps2[:, 0:F], onesb[:], sq[:, k * F:(k + 1) * F],
                             start=(k == 0), stop=(k == CH - 1))
        mean_bf = pool.tile([128, F], bf16, tag="ln_meanb", bufs=1)
        mean2 = pool.tile([128, F], f32, tag="ln_mean2", bufs=1)
        rstd_bf = pool.tile([128, F], bf16, tag="ln_rstdb", bufs=1)
        nc.vector.tensor_scalar(mean_bf[:], pm[:, 0:F], 1.0 / C, None, op0=AL.mult)
        nc.scalar.activation(mean2[:], pm[:, 0:F], AF.Square, scale=1.0 / C)
        nc.vector.scalar_tensor_tensor(mean2[:], ps2[:, 0:F], 1.0 / C, mean2[:],
                                       op0=AL.mult, op1=AL.subtract)
        # rstd = exp(-0.5*ln(var+eps))  (custom-DVE recip ops fail codegen here)
        nc.scalar.activation(mean2[:], mean2[:], AF.Ln, bias=eps_t[:, 0:1])
        nc.scalar.activation(rstd_bf[:], mean2[:], AF.Exp, scale=-0.5)
        tmp = pool.tile([128, F], bf16, tag="ln_tmp", bufs=1)
        for k in range(CH):
            nc.vector.tensor_tensor(tmp[:], xk(k), mean_bf[:], op=AL.subtract)
            nc.vector.tensor_tensor(
                xh[:, k * F:(k + 1) * F], tmp[:], rstd_bf[:], op=AL.mult)
        return xh

    def load_bias(dram_t, l, cols):
        bt = bpool.tile([128, cols], f32, tag=dram_t.name)
        nc.sync.dma_start(bt[:], dram_t[l])
        return bt

    # ------------------------------------------------------------ layers
    for l in range(DEPTH):
        F = F0 if l <= SEL else F1
        N = N0 if l <= SEL else N1
        mlens = [128, N - 128]

        qkvb = load_bias(d["qkvb_d"], l, 12)
        projb = load_bias(d["projb_d"], l, 6)

        # ---------------- phase A: LN1 + QKV + attention + proj ----------------
        wq = wA.tile([128, CH * 3 * C], bf16, tag="wqkv")
        nc.sync.dma_start(wq[:].rearrange("p (k n) -> p k n", k=CH), d["qkvw_d"][l].rearrange("(k p) n -> p k n", p=128))
        wpj = wA.tile([128, CH * C], bf16, tag="wproj")
        nc.sync.dma_start(wpj[:].rearrange("p (k n) -> p k n", k=CH), d["projw_d"][l].rearrange("(k p) n -> p k n", p=128))

        with tc.tile_pool(name="tA", bufs=1) as tA:
            for p in range(PAIRS):
                xh = layernorm(tA, xt[p], F, "ln1")
                qT = tA.tile([128, CH * F], bf16, tag="qT", bufs=2, name="qT")
                kT = tA.tile([128, CH * F], bf16, tag="kT", bufs=2, name="kT")
                for o in range(12):
                    ps = psA()
                    for k in range(CH):
                        nc.tensor.matmul(
                            ps[:, 0:F],
                            wq[:, k * 3 * C + o * 128: k * 3 * C + o * 128 + 128],
                            xh[:, k * F:(k + 1) * F],
                            start=(k == 0), stop=(k == CH - 1),
                        )
                    oc = o % CH
                    if o < CH:
                        # q evac on ScalarE (bias add; q-scale folded on host)
                        nc.scalar.add(qT[:, oc * F:(oc + 1) * F], ps[:, 0:F],
                                      qkvb[:, o:o + 1])
                    else:
                        nc.vector.tensor_scalar(
                            kT[:, oc * F:(oc + 1) * F], ps[:, 0:F],
                            qkvb[:, o:o + 1], None, op0=AL.add)

                # v in token-partition layout, per image: 2 t-chunks
                vto = [[None, None], [None, None]]
                for b in range(2):
                    for tchunk in range(2):
                        tlen = mlens[tchunk]
                        toff = b * N + tchunk * 128
                        vt = tA.tile([128, C], bf16, tag=f"v{b}{tchunk}", bufs=2)
                        vto[b][tchunk] = vt
                        for half in range(2):
                            ps = psA()
                            for k in range(CH):
                                nc.tensor.matmul(
                                    ps[0:tlen, 0:384],
                                    xh[:, k * F + toff: k * F + toff + tlen],
                                    wq[:, k * 3 * C + 2 * C + half * 384:
                                       k * 3 * C + 2 * C + half * 384 + 384],
                                    start=(k == 0), stop=(k == CH - 1),
                                )
                            nc.vector.tensor_copy(
                                vt[0:tlen, half * 384:(half + 1) * 384],
                                ps[0:tlen, 0:384])

                # attention, per head-pair hp; heads hh=0/1 pack into one PSUM
                # bank (odd head -> partitions 64:128 via auto col tile_position)
                oT = tA.tile([128, CH * F], bf16, tag="oT", bufs=2, name="oT")
                for hp in range(HD // 2):
                    qcol = hp * F
                    et = [[None, None], [None, None]]   # [hh][tchunk]
                    for hh in range(2):
                        qrow = hh * 64
                        pev = psD() if l == SEL else None
                        for tchunk in range(2):
                            tlen = mlens[tchunk]
                            ps_s = psS()
                            for b in range(2):
                                nc.tensor.matmul(
                                    ps_s[0:tlen, b * N:(b + 1) * N],
                                    kT[qrow:qrow + 64,
                                       qcol + b * N + tchunk * 128:
                                       qcol + b * N + tchunk * 128 + tlen],
                                    qT[qrow:qrow + 64, qcol + b * N: qcol + (b + 1) * N],
                                    start=True, stop=True,
                                )
                            if l == SEL:
                                rt = tA.tile([128, F], f32r, tag="rsb", bufs=2)
                                nc.vector.tensor_scalar(
                                    rt[0:tlen, 0:F], ps_s[0:tlen, 0:F],
                                    0.0, None, op0=AL.max)
                                nc.tensor.matmul(
                                    pev[0:1, 0:F], onesr[0:tlen, 0:1], rt[0:tlen, 0:F],
                                    start=(tchunk == 0), stop=(tchunk == 1),
                                )
                            ett = tA.tile([128, F], bf16, tag=f"et{hh}{tchunk}",
                                          bufs=2, name=f"et{hh}{tchunk}")
                            et[hh][tchunk] = ett
                            nc.scalar.activation(
                                ett[0:tlen, 0:F], ps_s[0:tlen, 0:F], AF.Exp)
                        if l == SEL:
                            ev1 = tA.tile([1, F], f32, tag="rsb", bufs=2)
                            nc.vector.tensor_scalar(
                                ev1[:], pev[0:1, 0:F], float(N), None, op0=AL.add)
                            nc.scalar.activation(ev1[:], ev1[:], AF.Ln)
                            nc.scalar.activation(ev1[:], ev1[:], AF.Exp, scale=-1.0)
                            if hp == 0 and hh == 0:
                                nc.vector.tensor_copy(unc[p][:, 0:F], ev1[:])
                            else:
                                nc.vector.tensor_tensor(
                                    unc[p][:, 0:F], ev1[:],
                                    unc[p][:, 0:F], op=AL.add)
                            if hp == HD // 2 - 1 and hh == 1:
                                # prefill this pair's U rows for the prune
                                for bb in range(2):
                                    nc.sync.dma_start(
                                        U[2 * p + bb:2 * p + bb + 1, :],
                                        unc[p][:, bb * N0 + 1:(bb + 1) * N0])
                    # softmax denominators for both heads in one bank
                    prs = psD()
                    for hh in range(2):
                        for tchunk in range(2):
                            tlen = mlens[tchunk]
                            nc.tensor.matmul(
                                prs[hh * 64:hh * 64 + 64, 0:F],
                                onesb[0:tlen, 0:64],
                                et[hh][tchunk][0:tlen, 0:F],
                                start=(tchunk == 0), stop=(tchunk == 1),
                            )
                    rsb = tA.tile([128, F], f32, tag="rsb", bufs=2)
                    nc.scalar.activation(rsb[:, 0:F], prs[:, 0:F], AF.Ln)
                    nc.scalar.activation(rsb[:, 0:F], rsb[:, 0:F], AF.Exp, scale=-1.0)
                    # AV per image, both heads into one bank
                    pav = psV()
                    for hh in range(2):
                        h = 2 * hp + hh
                        for b in range(2):
                            for tchunk in range(2):
                                tlen = mlens[tchunk]
                                nc.tensor.matmul(
                                    pav[hh * 64:hh * 64 + 64, b * N:(b + 1) * N],
                                    vto[b][tchunk][0:tlen, h * 64:h * 64 + 64],
                                    et[hh][tchunk][0:tlen, b * N:(b + 1) * N],
                                    start=(tchunk == 0), stop=(tchunk == 1),
                                )
                    # normalize + evacuate: one op per head pair
                    nc.vector.tensor_tensor(
                        oT[:, qcol:qcol + F], pav[:, 0:F], rsb[:, 0:F], op=AL.mult)

                # proj + residual (v-bias folded into projb on host)
                for co in range(CH):
                    ps = psA()
                    for k in range(CH):
                        nc.tensor.matmul(
                            ps[:, 0:F],
                            wpj[:, k * C + co * 128: k * C + co * 128 + 128],
                            oT[:, k * F:(k + 1) * F],
                            start=(k == 0), stop=(k == CH - 1),
                        )
                    nc.vector.scalar_tensor_tensor(
                        xt[p][:, co * F:(co + 1) * F],
                        ps[:, 0:F], projb[:, co:co + 1],
                        xt[p][:, co * F:(co + 1) * F].bitcast(f32),
                        op0=AL.add, op1=AL.add)

        tap(1 + 2 * l, xt[0], F)

        # ---------------- pruning (after layer-SEL attention residual) --------
        if l == SEL:
            _prune(nc, tc, xt, U, identb, identf, ltt, iota, psS, psD, d)

        F = F0 if l < SEL else F1

        fc1b = load_bias(d["fc1b_d"], l, 24)
        fc2b = load_bias(d["fc2b_d"], l, 6)

        # ---------------- phase B: LN2 + MLP in 4 quarters ---------------------
        with tc.tile_pool(name="tB", bufs=1) as tB:
            xh2 = [layernorm(tB, xt[p], F, f"ln2_{p}") for p in range(PAIRS)]
            h1 = [tB.tile([128, CH * F], bf16, tag=f"h1_{p}", name=f"h1_{p}") for p in range(PAIRS)]
            for q in range(4):
                w1 = wB.tile([128, CH * C], bf16, tag="wfc1", bufs=2)
                nc.sync.dma_start(
                    w1[:].rearrange("p (k n) -> p k n", k=CH),
                    d["fc1w_d"][l][:, q * C:(q + 1) * C].rearrange("(k p) n -> p k n", p=128))
                w2 = wB.tile([128, CH * C], bf16, tag="wfc2", bufs=2)
                nc.sync.dma_start(
                    w2[:].rearrange("p (k n) -> p k n", k=CH),
                    d["fc2w_d"][l][q * C:(q + 1) * C, :].rearrange("(k p) n -> p k n", p=128))
                for p in range(PAIRS):
                    for co in range(CH):
                        ps = psA()
                        for k in range(CH):
                            nc.tensor.matmul(
                                ps[:, 0:F],
                                w1[:, k * C + co * 128: k * C + co * 128 + 128],
                                xh2[p][:, k * F:(k + 1) * F],
                                start=(k == 0), stop=(k == CH - 1),
                            )
                        nc.scalar.activation(
                            h1[p][:, co * F:(co + 1) * F], ps[:, 0:F],
                            AF.Gelu, bias=fc1b[:, q * CH + co:q * CH + co + 1])
                    for co in range(CH):
                        ps = psA()
                        for k in range(CH):
                            nc.tensor.matmul(
                                ps[:, 0:F],
                                w2[:, k * C + co * 128: k * C + co * 128 + 128],
                                h1[p][:, k * F:(k + 1) * F],
                                start=(k == 0), stop=(k == CH - 1),
                            )
                        if q == 0:
                            nc.vector.scalar_tensor_tensor(
                                xt[p][:, co * F:(co + 1) * F],
                                ps[:, 0:F], fc2b[:, co:co + 1],
                                xt[p][:, co * F:(co + 1) * F].bitcast(f32),
                                op0=AL.add, op1=AL.add)
                        else:
                            nc.vector.tensor_tensor(
                                xt[p][:, co * F:(co + 1) * F],
                                ps[:, 0:F],
                                xt[p][:, co * F:(co + 1) * F].bitcast(f32),
                                op=AL.add)
        tap(2 + 2 * l, xt[0], F)

    # ------------------------------------------------------------ head
    with tc.tile_pool(name="whead", bufs=1) as wh, tc.tile_pool(name="thead", bufs=1) as th:
        clsT = th.tile([128, CH * B_CORE], bf16, tag="clsT")
        for p in range(PAIRS):
            for b in range(2):
                for k in range(CH):
                    nc.vector.tensor_copy(
                        clsT[:, k * B_CORE + 2 * p + b: k * B_CORE + 2 * p + b + 1],
                        xt[p][:, k * F1 + b * N1: k * F1 + b * N1 + 1].bitcast(f32))
        xhc = layernorm(th, clsT, B_CORE, "lnf", xf32=False)
        hw = wh.tile([128, CH * NCLS], bf16, tag="hw")
        nc.sync.dma_start(hw[:].rearrange("p (k n) -> p k n", k=CH), d["headw_d"].rearrange("(k p) n -> p k n", p=128))
        hb = wh.tile([NCLS, 1], f32, tag="hb")
        nc.sync.dma_start(hb[:], d["headb_d"][:])
        ps = psD()
        for k in range(CH):
            nc.tensor.matmul(
                ps[0:NCLS, 0:B_CORE],
                hw[:, k * NCLS:(k + 1) * NCLS],
                xhc[:, k * B_CORE:(k + 1) * B_CORE],
                start=(k == 0), stop=(k == CH - 1),
            )
        lt = th.tile([NCLS, B_CORE], f32, tag="logits")
        nc.vector.tensor_scalar(lt[:], ps[0:NCLS, 0:B_CORE], hb[:, 0:1], None, op0=AL.add)
        nc.sync.dma_start(d["out_d"][:], lt[:])

    es.close()


def _prune(nc, tc, xt, U, identb, identf, ltt, iota, psS, psD, d):
    """Keep the K_KEEP lowest-uncertainty image tokens (drop the N_DROP
    highest), append mean of dropped; rewrite x in-place to [128, CH*F1].
    U rows were prefilled (via DMA) during phase A."""
    f32, bf16 = dt.float32, dt.bfloat16
    jl = [128, 68]          # img-token chunk lengths (196 = 128 + 68)
    with tc.tile_pool(name="tprune", bufs=1) as tp:
        # drop mask: top-N_DROP largest per row (unc ~ 1, min_val 0 is safe;
        # scale first so the min(.,1) mask threshold is safe)
        nc.vector.tensor_scalar(U[:], U[:], 100.0, None, op0=AL.mult)
        work = tp.tile([B_CORE, 196], f32, tag="work")
        mx = tp.tile([B_CORE, 8], f32, tag="mx")
        cur = U
        for k_on in range(0, N_DROP, 8):
            nfind = min(k_on + 8, N_DROP) - k_on
            nc.vector.max(out=mx[:], in_=cur[:])
            if nfind < 8:
                nc.vector.memset(mx[:, nfind:], 0.0)
            nc.vector.match_replace(out=work[:], in_to_replace=mx[:],
                                    in_values=cur[:], imm_value=0.0)
            cur = work
        nc.vector.tensor_sub(work[:], U[:], work[:])
        nc.vector.tensor_scalar_min(work[:], work[:], 1.0)   # drop mask {0,1}
        keep = tp.tile([B_CORE, 196], f32, tag="keep")
        nc.vector.tensor_scalar(keep[:], work[:], -1.0, 1.0, op0=AL.mult, op1=AL.add)
        if d.get("dbgp_d") is not None:
            nc.sync.dma_start(d["dbgp_d"][0][0:8, :], U[:])
            nc.sync.dma_start(d["dbgp_d"][1][0:8, :], keep[:])

        # keepT chunks via PE transpose (bf16 for the ranks matmul vs ltt)
        keepT = [tp.tile([128, B_CORE], bf16, tag=f"keepT{i}", name=f"keepT{i}") for i in range(2)]
        for i in range(2):
            pt = psS()
            nc.tensor.transpose(pt[0:jl[i], 0:B_CORE],
                                keep[:, i * 128:i * 128 + jl[i]],
                                identf[0:B_CORE, 0:B_CORE])
            nc.vector.tensor_copy(keepT[i][0:jl[i], :], pt[0:jl[i], 0:B_CORE])
        # ranks = inclusive cumsum of keep via lower-triangular ones matmul
        prk = psD()
        for i in range(2):
            nc.tensor.matmul(
                prk[0:B_CORE, 0:196], keepT[i][0:jl[i], :],
                ltt[0:jl[i], i * 196:(i + 1) * 196],
                start=(i == 0), stop=(i == 1))
        ranks = tp.tile([B_CORE, 196], f32, tag="ranks")
        nc.vector.tensor_copy(ranks[:], prk[0:B_CORE, 0:196])
        if d.get("dbgp_d") is not None:
            nc.sync.dma_start(d["dbgp_d"][2][0:8, :], ranks[:])
        # target col t = keep*rank + (1-keep)*138 ; weight w = keep + (1-keep)/59
        tcol = tp.tile([B_CORE, 196], f32, tag="tcol")
        nc.vector.tensor_tensor(tcol[:], ranks[:], keep[:], op=AL.mult)
        nc.vector.scalar_tensor_tensor(tcol[:], keep[:], -float(N1 - 1), tcol[:],
                                       op0=AL.mult, op1=AL.add)
        nc.vector.tensor_scalar(tcol[:], tcol[:], float(N1 - 1), None, op0=AL.add)
        wcol = tp.tile([B_CORE, 196], f32, tag="wcol")
        nc.vector.tensor_scalar(wcol[:], keep[:], float((N_DROP - 1) / N_DROP),
                                1.0 / N_DROP, op0=AL.mult, op1=AL.add)
        tT = [tp.tile([128, B_CORE], f32, tag=f"tT{i}", name=f"tT{i}") for i in range(2)]
        wT = [tp.tile([128, B_CORE], f32, tag=f"wT{i}", name=f"wT{i}") for i in range(2)]
        for i in range(2):
            pt = psS()
            nc.tensor.transpose(pt[0:jl[i], 0:B_CORE],
                                tcol[:, i * 128:i * 128 + jl[i]],
                                identf[0:B_CORE, 0:B_CORE])
            nc.vector.tensor_copy(tT[i][0:jl[i], :], pt[0:jl[i], 0:B_CORE])
            pt2 = psS()
            nc.tensor.transpose(pt2[0:jl[i], 0:B_CORE],
                                wcol[:, i * 128:i * 128 + jl[i]],
                                identf[0:B_CORE, 0:B_CORE])
            nc.vector.tensor_copy(wT[i][0:jl[i], :], pt2[0:jl[i], 0:B_CORE])

        # per pair: transpose old x (img tokens only, cls-skipped so chunks
        # align with P), cls copies, then one-hot gather matmul, in place.
        for p in range(PAIRS):
            xa = xt[p]
            xtok = {}
            for b in range(2):
                for i in range(2):
                    tlen = jl[i]
                    xk = tp.tile([128, CH * 128], bf16, tag=f"xtok{b}{i}")
                    xtok[(b, i)] = xk
                    for k in range(CH):
                        pt = psS()
                        nc.tensor.transpose(
                            pt[0:tlen, 0:128],
                            xa[:, k * F0 + b * N0 + 1 + i * 128:
                               k * F0 + b * N0 + 1 + i * 128 + tlen].bitcast(f32),
                            identf[:])
                        nc.vector.tensor_copy(xk[0:tlen, k * 128:(k + 1) * 128],
                                              pt[0:tlen, 0:128])
            for b in range(2):
                for k in range(CH):
                    nc.vector.tensor_copy(
                        xa[:, k * F1 + b * N1: k * F1 + b * N1 + 1],
                        xa[:, k * F0 + b * N0: k * F0 + b * N0 + 1])
            for b in range(2):
                img = 2 * p + b
                P = [tp.tile([128, N1 - 1], bf16, tag=f"P{i}", name=f"P{i}") for i in range(2)]
                for i in range(2):
                    nc.vector.tensor_scalar(
                        P[i][0:jl[i], :], iota[0:jl[i], :],
                        tT[i][0:jl[i], img:img + 1], wT[i][0:jl[i], img:img + 1],
                        op0=AL.is_equal, op1=AL.mult)
                for k in range(CH):
                    pg = psD()
                    for i in range(2):
                        nc.tensor.matmul(
                            pg[0:128, 0:N1 - 1],
                            xtok[(b, i)][0:jl[i], k * 128:(k + 1) * 128],
                            P[i][0:jl[i], :],
                            start=(i == 0), stop=(i == 1))
                    nc.vector.tensor_copy(
                        xa[:, k * F1 + b * N1 + 1: k * F1 + b * N1 + N1],
                        pg[0:128, 0:N1 - 1])


# ------------------------------------------------------------------- host
def _host_pack(inputs):
    """Fold LN affines into weights, pre-transpose, pre-extract patches,
    fold q-scale into q weights and v-bias into proj bias, cast to bf16."""
    f = np.float32
    inp = {k: np.asarray(v, f) for k, v in inputs.items()}
    out = {}

    imgs = inp['inputs']
    B = imgs.shape[0]
    x = imgs.reshape(B, 3, 14, 16, 14, 16).transpose(0, 2, 4, 1, 3, 5).reshape(B, 196, 768)
    out['patchesT_full'] = np.ascontiguousarray(
        x.transpose(2, 0, 1).reshape(768, B * 196)).astype(BF16)

    posC = inp['pos_embed'][0].copy()
    posC[0] += inp['cls_token'][0, 0]
    posC[1:] += inp['patch_b'][None, :]
    out['posCT'] = np.ascontiguousarray(posC.T)

    out['patch_wT'] = np.ascontiguousarray(inp['patch_w'].reshape(C, -1).T).astype(BF16)

    qkv_wT = np.empty((DEPTH, C, 3 * C), f)
    qkv_bL = np.empty((DEPTH, 128, 12), f)
    proj_wT = np.empty((DEPTH, C, C), f)
    proj_bL = np.empty((DEPTH, 128, 6), f)
    fc1_wT = np.empty((DEPTH, C, 4 * C), f)
    fc1_bL = np.empty((DEPTH, 128, 24), f)
    fc2_wT = np.empty((DEPTH, 4 * C, C), f)
    fc2_bL = np.empty((DEPTH, 128, 6), f)
    for l in range(DEPTH):
        w1 = inp['qkv_w'][l] * inp['ln1_g'][l][None, :]
        b1 = inp['qkv_b'][l] + inp['qkv_w'][l] @ inp['ln1_b'][l]
        w1 = w1.copy()
        w1[:C] *= SCALE          # q-scale folded into q weights
        b1 = b1.copy()
        b1[:C] *= SCALE
        qkv_wT[l] = w1.T
        qkv_bL[l] = b1[:2 * C].reshape(12, 128).T
        proj_wT[l] = inp['proj_w'][l].T
        # v-bias folded into proj bias: o = AV/d + b_v  =>  Wp@o + bp
        bp = inp['proj_b'][l] + inp['proj_w'][l] @ b1[2 * C:]
        proj_bL[l] = bp.reshape(6, 128).T
        wf1 = inp['fc1_w'][l] * inp['ln2_g'][l][None, :]
        bf1 = inp['fc1_b'][l] + inp['fc1_w'][l] @ inp['ln2_b'][l]
        fc1_wT[l] = wf1.T
        fc1_bL[l] = bf1.reshape(24, 128).T
        fc2_wT[l] = inp['fc2_w'][l].T
        fc2_bL[l] = inp['fc2_b'][l].reshape(6, 128).T
    out.update(qkv_wT=qkv_wT.astype(BF16), qkv_bL=qkv_bL,
               proj_wT=proj_wT.astype(BF16), proj_bL=proj_bL,
               fc1_wT=fc1_wT.astype(BF16), fc1_bL=fc1_bL,
               fc2_wT=fc2_wT.astype(BF16), fc2_bL=fc2_bL)

    hw = inp['head_w'] * inp['norm_g'][None, :]
    hb = inp['head_b'] + inp['head_w'] @ inp['norm_b']
    out['headT'] = np.ascontiguousarray(hw.T).astype(BF16)
    out['head_bL'] = np.ascontiguousarray(hb.reshape(NCLS, 1))

    out['identb'] = np.eye(128, dtype=f).astype(BF16)
    out['identf'] = np.eye(128, dtype=f)
    out['onesb'] = np.ones((128, 128), f).astype(BF16)
    out['onesr'] = np.ones((128, 128), f)   # fp32r tile; bits == fp32
    out['iota'] = np.tile(np.arange(1, N1, dtype=f), (128, 1)).astype(BF16)
    out['LT'] = (np.arange(196)[:, None] <= np.arange(196)[None, :]).astype(f).astype(BF16)
    return out


_BUILT = None


def kernel(**inputs):
    global _BUILT
    host = _host_pack(inputs)
    if _BUILT is None:
        nc = build_nc()
        split_excess_waits(nc)
        _BUILT = nc
    nc = _BUILT

    shared_keys = ['posCT', 'patch_wT', 'qkv_wT', 'qkv_bL', 'proj_wT', 'proj_bL',
                   'fc1_wT', 'fc1_bL', 'fc2_wT', 'fc2_bL', 'headT', 'head_bL',
                   'identb', 'identf', 'onesb', 'onesr', 'iota', 'LT']
    in_maps = []
    for c in range(NCORES):
        m = {k: host[k] for k in shared_keys}
        m['patchesT'] = np.ascontiguousarray(
            host['patchesT_full'][:, c * B_CORE * 196:(c + 1) * B_CORE * 196])
        in_maps.append(m)

    trace = bool(os.environ.get("BASS_VIT_TRACE"))
    res = run_bass_kernel_spmd(nc, in_maps, core_ids=list(range(NCORES)), trace=trace)
    if trace:
        print(f"HW exec time: {res.exec_time_ns} ns (mean {res.mean_exec_time_ns})")
        kernel.last_exec_time_ns = res.exec_time_ns
        kernel.last_res = res

    out = np.concatenate([res.results[c]["logitsT"].T for c in range(NCORES)],
                         axis=0).astype(np.float32)
    if os.environ.get("BASS_VIT_DEBUG_LAYER", ""):
        kernel.last_dbg = [res.results[c].get("dbg") for c in range(NCORES)]
        kernel.last_dbgp = [res.results[c].get("dbgp") for c in range(NCORES)]
    return out

